# revision 1
# baseline (speedup 1.0000x reference)
"""Trainium2 Bass kernel for nn_NeuralSurface (8-layer MLP SDF with harmonic
embedding + skip concat), data-parallel over 8 NeuronCores.

Layout strategy: activations kept transposed in SBUF ([features, points]),
weights stationary fp16, PE matmuls K/M-chunked to 128. Harmonic sin/cos via
ScalarE Sin LUT after DVE range reduction to [-pi, pi] (magic-number
round-to-nearest). ReLU+bias split between ScalarE (activation Relu w/ bias)
and VectorE (tensor_scalar add+max) reading PSUM. n-tiles processed in pairs
so the PE always has independent matmul work while ReLUs complete.
"""

import numpy as np

import concourse.bacc as bacc
import concourse.mybir as mybir
import concourse.tile as tile
from concourse.bass_utils import run_bass_kernel_spmd

AF = mybir.ActivationFunctionType
ALU = mybir.AluOpType
F32 = mybir.dt.float32
F16 = mybir.dt.float16

N_CORES = 8
N = 262144
NPC = N // N_CORES  # 32768 points per core
NT = 512  # points per n-tile (PSUM bank / fp32 moving-operand limit)
PAIRS = NPC // (2 * NT)  # 32
H = 256
E = 39
NHARM = 6
TWO_PI = float(2.0 * np.pi)
MAGIC = float(1.5 * 2.0**23)  # round-to-nearest via (x + M) - M

# ReLU engine split: half 0 -> ACT, half 1 -> DVE (even split; each PSUM
# pair drains through two engines in parallel).
DVE_RELU = {(li, 1): True for li in range(8)}

_CACHED = {}


def _build():
    nc = bacc.Bacc("TRN2")

    rep6 = nc.dram_tensor("rep6", [128, NPC], F32, kind="ExternalInput").ap()
    ptsh = nc.dram_tensor("ptsh", [3, NPC], F16, kind="ExternalInput").ap()
    w0h = nc.dram_tensor("w0h", [128, H], F16, kind="ExternalInput").ap()
    wkh = {
        i: nc.dram_tensor(f"w{i}h", [H, H], F16, kind="ExternalInput").ap()
        for i in (1, 2, 3, 5, 6, 7)
    }
    w4eh = nc.dram_tensor("w4eh", [128, H], F16, kind="ExternalInput").ap()
    w4ah = nc.dram_tensor("w4ah", [128, H], F16, kind="ExternalInput").ap()
    w4bh = nc.dram_tensor("w4bh", [128, H], F16, kind="ExternalInput").ap()
    wsdfh = nc.dram_tensor("wsdfh", [H, 1], F16, kind="ExternalInput").ap()
    bmat = nc.dram_tensor("bmat", [128, 16], F32, kind="ExternalInput").ap()
    bsdf1 = nc.dram_tensor("bsdf1", [128, 1], F32, kind="ExternalInput").ap()
    # 2-D output (1-D ExternalOutput tensors fail NEFF load under bass2jax)
    out_o = nc.dram_tensor("out_o", [NPC // NT, NT], F32, kind="ExternalOutput").ap()

    with tile.TileContext(nc) as tc:
        with (
            tc.tile_pool(name="wp", bufs=1) as wp,
            tc.tile_pool(name="ep", bufs=4) as ep,
            tc.tile_pool(name="hp", bufs=4) as hp,
            tc.tile_pool(name="op", bufs=4) as op_,
            tc.tile_pool(name="pp", bufs=6, space="PSUM") as pp,
            tc.tile_pool(name="pf", bufs=1, space="PSUM") as pf,
        ):
            # ---- one-time weight / const loads ----
            w0s = wp.tile_from(w0h, name="w0s")  # [39, 256]
            wks = {
                i: (
                    wp.tile_from(wkh[i][0:128, :], name=f"wks{i}a"),
                    wp.tile_from(wkh[i][128:256, :], name=f"wks{i}b"),
                )
                for i in (1, 2, 3, 5, 6, 7)
            }
            w4es = wp.tile_from(w4eh, name="w4es")  # [128, 256] K-padded
            w4as = wp.tile_from(w4ah, name="w4as")  # [128, 256]
            w4bs = wp.tile_from(w4bh, name="w4bs")
            wsdf_a = wp.tile_from(wsdfh[0:128, :], name="wsdf_a")  # [128, 1]
            wsdf_b = wp.tile_from(wsdfh[128:256, :], name="wsdf_b")
            bms = wp.tile_from(bmat, name="bms")  # [128, 16]
            bsdfs = wp.tile_from(bsdf1, name="bsdfs")  # [1, 1]
            zcol = wp.tile([128, 1], F32, name="zcol")
            nc.vector.memset(zcol, 0.0)

            def wchunk(i, k, m):
                # lhsT [128, 128] slice: layer i, K-chunk k, M-half m
                return wks[i][k][:, bass_ts(m, 128)]

            for p in range(PAIRS):
                s = p * 2 * NT  # start point index of the pair (A at s, B at s+NT)
                W = 2 * NT  # pair-wide free size

                # ---- embedding (pair-wide, [128, 1024] ops) ----
                # rep6 rows carry t0 = x*2^j/(2pi) + phase (host-precomputed
                # exact scaling); rows 36:128 are zero -> Sin gives 0, so emb
                # is K-padded to 128 for free (full-K weight loads on PE).
                t0 = ep.tile([128, W], F32, tag="t0")
                nc.sync.dma_start(out=t0, in_=rep6[:, s:s + W])
                rr = ep.tile([128, W], F32, tag="rr")
                nc.vector.tensor_scalar(rr, t0, MAGIC, MAGIC, op0=ALU.add, op1=ALU.subtract)
                ys = ep.tile([128, W], F32, tag="ys")
                nc.vector.tensor_tensor(out=ys, in0=t0, in1=rr, op=ALU.subtract)

                emb = ep.tile([128, W], F16, tag="emb")
                nc.scalar.activation(emb, ys, AF.Sin, bias=zcol, scale=TWO_PI)
                nc.sync.dma_start(out=emb[36:39, :], in_=ptsh[:, s:s + W])

                # ---- MLP layers ----
                # h tile layout: [128, 4*NT]: A-half0, A-half1, B-half0, B-half1
                h_prev = None
                h3 = None
                for li in range(8):
                    h = hp.tile([128, 4 * NT], F16, tag="h")
                    # chunks: list of (weight tile [128,256], rhs per half_x)
                    if li == 0:
                        chunks = [(w0s, lambda hx: emb[:, bass_ts(hx, NT)])]
                    elif li == 4:
                        chunks = [
                            (w4es, lambda hx: emb[:, bass_ts(hx, NT)]),
                            (w4as, lambda hx, hp3=h3: hp3[:, bass_ts(2 * hx, NT)]),
                            (w4bs, lambda hx, hp3=h3: hp3[:, bass_ts(2 * hx + 1, NT)]),
                        ]
                    else:
                        chunks = [
                            (wks[li][0], lambda hx, hp_=h_prev: hp_[:, bass_ts(2 * hx, NT)]),
                            (wks[li][1], lambda hx, hp_=h_prev: hp_[:, bass_ts(2 * hx + 1, NT)]),
                        ]
                    ps = {(hx, m): pp.tile([128, NT], F32, tag="mm", name="psmm")
                          for hx in range(2) for m in range(2)}
                    last = len(chunks) - 1
                    for hx in range(2):
                        for m in range(2):
                            for ci, (wt, rhs) in enumerate(chunks):
                                nc.tensor.matmul(
                                    ps[(hx, m)], wt[:, bass_ts(m, 128)], rhs(hx),
                                    start=(ci == 0), stop=(ci == last),
                                )
                    # ReLU + bias -> h
                    for half_x in range(2):
                        for m in range(2):
                            dst = h[:, bass_ts(2 * half_x + m, NT)]
                            bias_ap = bms[:, li * 2 + m:li * 2 + m + 1]
                            if DVE_RELU.get((li, m), False):
                                nc.vector.tensor_scalar(
                                    dst, ps[(half_x, m)], bias_ap, 0.0,
                                    op0=ALU.add, op1=ALU.max,
                                )
                            else:
                                nc.scalar.activation(
                                    dst, ps[(half_x, m)], AF.Relu, bias=bias_ap,
                                )
                    if li == 3:
                        h3 = h
                    h_prev = h

                # ---- final SDF layer (M=1), col-group packed: A at array
                # col 0, B at array col 32 -> the two tiles' matmuls overlap
                # on the PE. Separate PSUM banks (same-bank dual accumulation
                # groups + DVE read crashed the exec unit).
                psfa = pf.tile([1, NT], F32, tag="finA")
                psfb_t = pf.tile([33, NT], F32, tag="finB")
                psfb = psfb_t[32:33, :]
                nc.tensor.matmul(
                    psfa, wsdf_a, h_prev[:, bass_ts(0, NT)],
                    start=True, stop=False, tile_position=(0, 0),
                    skip_group_check=True,
                )
                nc.tensor.matmul(
                    psfb, wsdf_a, h_prev[:, bass_ts(2, NT)],
                    start=True, stop=False, tile_position=(0, 32),
                    skip_group_check=True,
                )
                nc.tensor.matmul(
                    psfa, wsdf_b, h_prev[:, bass_ts(1, NT)],
                    start=False, stop=True, tile_position=(0, 0),
                    skip_group_check=True,
                )
                nc.tensor.matmul(
                    psfb, wsdf_b, h_prev[:, bass_ts(3, NT)],
                    start=False, stop=True, tile_position=(0, 32),
                    skip_group_check=True,
                )
                oa = op_.tile([1, NT], F32, tag="oa")
                nc.scalar.activation(oa, psfa, AF.Identity, bias=bsdfs[0:1, 0:1])
                ob = op_.tile([1, NT], F32, tag="ob")
                nc.scalar.activation(ob, psfb, AF.Identity, bias=bsdfs[0:1, 0:1])
                nc.sync.dma_start(out=out_o[2 * p:2 * p + 1, :], in_=oa)
                nc.sync.dma_start(out=out_o[2 * p + 1:2 * p + 2, :], in_=ob)
    nc.compile()
    return nc


def bass_ts(i, size):
    return slice(i * size, (i + 1) * size)


def _prep_maps(points, ws, bs, wsdf, bsdf):
    pts = np.ascontiguousarray(points, dtype=np.float32).reshape(N, 3)
    freqs = (2.0 ** np.arange(NHARM)).astype(np.float32)
    fcol18 = (np.repeat(freqs[None, :], 3, axis=0).reshape(18, 1) / TWO_PI).astype(
        np.float32
    )

    bmat = np.zeros((128, 16), dtype=np.float32)
    for i in range(8):
        for m in range(2):
            bmat[:, i * 2 + m] = bs[i][m * 128:(m + 1) * 128]

    w0p = np.zeros((128, H), dtype=np.float16)
    w0p[0:E, :] = ws[0].astype(np.float16)
    w4ep = np.zeros((128, H), dtype=np.float16)
    w4ep[0:E, :] = ws[4][0:E, :].astype(np.float16)
    common = {
        "w0h": w0p,
        "w4eh": w4ep,
        "w4ah": ws[4][E:E + 128, :].astype(np.float16),
        "w4bh": ws[4][E + 128:E + 256, :].astype(np.float16),
        "wsdfh": wsdf.astype(np.float16),
        "bmat": bmat,
        "bsdf1": np.full((128, 1), float(np.ravel(bsdf)[0]), dtype=np.float32),
    }
    for i in (1, 2, 3, 5, 6, 7):
        common[f"w{i}h"] = ws[i].astype(np.float16)

    in_maps = []
    for c in range(N_CORES):
        sl = pts[c * NPC:(c + 1) * NPC]  # [NPC, 3]
        ptsT = np.ascontiguousarray(sl.T)  # [3, NPC]
        rep3 = np.repeat(ptsT, NHARM, axis=0)  # [18, NPC]
        t18 = rep3 * fcol18  # x * 2^j / (2pi), exact fp32 scaling
        rep6 = np.zeros((128, NPC), dtype=np.float32)
        rep6[0:18], rep6[18:36] = t18, t18 + np.float32(0.25)
        m = dict(common)
        m["rep6"] = rep6
        m["ptsh"] = ptsT.astype(np.float16)
        in_maps.append(m)
    return in_maps


def kernel(
    points, w0, b0, w1, b1, w2, b2, w3, b3, w4, b4, w5, b5, w6, b6, w7, b7,
    wsdf, bsdf,
):
    ws = [np.asarray(w, dtype=np.float32) for w in (w0, w1, w2, w3, w4, w5, w6, w7)]
    bs = [np.asarray(b, dtype=np.float32) for b in (b0, b1, b2, b3, b4, b5, b6, b7)]
    in_maps = _prep_maps(
        np.asarray(points), ws, bs,
        np.asarray(wsdf, dtype=np.float32), np.asarray(bsdf, dtype=np.float32),
    )

    if "nc" not in _CACHED:
        _CACHED["nc"] = _build()
    nc = _CACHED["nc"]

    res = run_bass_kernel_spmd(nc, in_maps, core_ids=list(range(N_CORES)))
    out = np.concatenate(
        [res.results[c]["out_o"] for c in range(N_CORES)], axis=0
    ).reshape(N, 1).astype(np.float32)
    return out



# revision 5
# speedup vs baseline: 1.0216x; 1.0216x over previous
"""Trainium2 Bass kernel for nn_NeuralSurface (8-layer MLP SDF with harmonic
embedding + skip concat), data-parallel over 8 NeuronCores.

Layout strategy: activations kept transposed in SBUF ([features, points]),
weights stationary fp16, PE matmuls K/M-chunked to 128. Harmonic sin/cos via
ScalarE Sin LUT after DVE range reduction to [-pi, pi] (magic-number
round-to-nearest). ReLU+bias split between ScalarE (activation Relu w/ bias)
and VectorE (tensor_scalar add+max) reading PSUM. n-tiles processed in pairs
so the PE always has independent matmul work while ReLUs complete.

v2: single packed weight tensor DMA'd in 3 chunks on the idle GpSimd queue
(vs 22 serial 600ns DIRECT2D issues on sync), pair-0 embedding computed in
NT-halves to cut first-matmul latency, PE warmup matmuls to finish the DVFS
p-state ramp before real work, SDF matmuls on plain PSUM tiles (the col-packed
tile_position variant measured 279ns vs 216ns per matmul).
"""

import numpy as np

import concourse.bacc as bacc
import concourse.mybir as mybir
import concourse.tile as tile
from concourse.bass_utils import run_bass_kernel_spmd

AF = mybir.ActivationFunctionType
ALU = mybir.AluOpType
F32 = mybir.dt.float32
F16 = mybir.dt.float16

N_CORES = 8
N = 262144
NPC = N // N_CORES  # 32768 points per core
NT = 512  # points per n-tile (PSUM bank / fp32 moving-operand limit)
PAIRS = NPC // (2 * NT)  # 32
H = 256
E = 39
NHARM = 6
TWO_PI = float(2.0 * np.pi)
MAGIC = float(1.5 * 2.0**23)  # round-to-nearest via (x + M) - M

# packed weight tensor column offsets ([128, WCOLS] fp16; K on partitions)
OFF_W0 = 0
_K_LAYERS = (1, 2, 3, 5, 6, 7)
OFF_WK = {li: 256 + j * 512 for j, li in enumerate(_K_LAYERS)}  # ka, kb halves
OFF_W4E = 256 + 6 * 512  # 3328
OFF_W4A = OFF_W4E + 256
OFF_W4B = OFF_W4A + 256
OFF_SDF = OFF_W4B + 256  # 2 cols: wsdf K-halves a, b
WCOLS = OFF_SDF + 2  # 4098

N_WARMUP = 6  # PE p-state ramp matmuls before real work

# ReLU engine split: half 0 -> ACT, half 1 -> DVE (even split; each PSUM
# pair drains through two engines in parallel).
DVE_RELU = {(li, 1): True for li in range(8)}

_CACHED = {}


def _build():
    nc = bacc.Bacc("TRN2")

    rep6 = nc.dram_tensor("rep6", [128, NPC], F32, kind="ExternalInput").ap()
    ptsh = nc.dram_tensor("ptsh", [3, NPC], F16, kind="ExternalInput").ap()
    wpack = nc.dram_tensor("wpack", [128, WCOLS], F16, kind="ExternalInput").ap()
    wwarm = nc.dram_tensor("wwarm", [128, 512], F16, kind="ExternalInput").ap()
    bmat = nc.dram_tensor("bmat", [128, 16], F32, kind="ExternalInput").ap()
    bsdf1 = nc.dram_tensor("bsdf1", [128, 1], F32, kind="ExternalInput").ap()
    # 2-D output (1-D ExternalOutput tensors fail NEFF load under bass2jax)
    out_o = nc.dram_tensor("out_o", [NPC // NT, NT], F32, kind="ExternalOutput").ap()

    with tile.TileContext(nc) as tc:
        with (
            tc.tile_pool(name="wp", bufs=1) as wp,
            tc.tile_pool(name="ep", bufs=4) as ep,
            tc.tile_pool(name="hp", bufs=4) as hp,
            tc.tile_pool(name="op", bufs=4) as op_,
            tc.tile_pool(name="pp", bufs=6, space="PSUM") as pp,
            tc.tile_pool(name="pf", bufs=1, space="PSUM") as pf,
        ):
            # ---- one-time weight / const loads ----
            # warmup weights on the scalar queue, packed weights in 3 chunks
            # on the gpsimd queue: both idle at boot, so the sync queue's
            # first issue is pair-0's rep6 (the embedding critical path).
            wws = wp.tile([128, 512], F16, name="wws")
            nc.scalar.dma_start(out=wws, in_=wwarm)
            wps = wp.tile([128, WCOLS], F16, name="wps")
            nc.gpsimd.dma_start(out=wps[:, 0:512], in_=wpack[:, 0:512])
            nc.gpsimd.dma_start(out=wps[:, 512:2304], in_=wpack[:, 512:2304])
            nc.gpsimd.dma_start(out=wps[:, 2304:WCOLS], in_=wpack[:, 2304:WCOLS])
            bms = wp.tile_from(bmat, name="bms")  # [128, 16]
            bsdfs = wp.tile_from(bsdf1, name="bsdfs")  # [1, 1]
            zcol = wp.tile([128, 1], F32, name="zcol")
            nc.vector.memset(zcol, 0.0)

            w0s = wps[:, OFF_W0:OFF_W0 + 256]  # [128(39), 256]
            wks = {
                li: (
                    wps[:, OFF_WK[li]:OFF_WK[li] + 256],
                    wps[:, OFF_WK[li] + 256:OFF_WK[li] + 512],
                )
                for li in _K_LAYERS
            }
            w4es = wps[:, OFF_W4E:OFF_W4E + 256]
            w4as = wps[:, OFF_W4A:OFF_W4A + 256]
            w4bs = wps[:, OFF_W4B:OFF_W4B + 256]
            wsdf_a = wps[:, OFF_SDF:OFF_SDF + 1]  # [128, 1]
            wsdf_b = wps[:, OFF_SDF + 1:OFF_SDF + 2]

            # ---- PE p-state warmup: dummy matmuls, no consumers ----
            # shares the "finA" PSUM bank (pair-0's SDF writes it much later)
            pwt = pf.tile([128, NT], F32, tag="finA", name="pwt")
            for _ in range(N_WARMUP):
                nc.tensor.matmul(pwt, wws[:, 0:128], wws, start=True, stop=True)

            for p in range(PAIRS):
                s = p * 2 * NT  # start point index of the pair (A at s, B at s+NT)
                W = 2 * NT  # pair-wide free size

                # ---- embedding (pair-wide, [128, 1024] ops) ----
                # rep6 rows carry t0 = x*2^j/(2pi) + phase (host-precomputed
                # exact scaling); rows 36:128 are zero -> Sin gives 0, so emb
                # is K-padded to 128 for free (full-K weight loads on PE).
                # Pair 0 is the kernel-start critical path: compute it in
                # NT-halves so the first L0 matmul starts one half earlier.
                emb = ep.tile([128, W], F16, tag="emb")
                halves = (0, NT) if p == 0 else (0,)
                hw_ = NT if p == 0 else W
                t0 = ep.tile([128, W], F32, tag="t0")
                nc.sync.dma_start(out=t0, in_=rep6[:, s:s + W])
                for hs in halves:
                    rr = ep.tile([128, hw_], F32, tag="rr")
                    nc.vector.tensor_scalar(
                        rr, t0[:, hs:hs + hw_], MAGIC, MAGIC,
                        op0=ALU.add, op1=ALU.subtract,
                    )
                    ys = ep.tile([128, hw_], F32, tag="ys")
                    nc.vector.tensor_tensor(
                        out=ys, in0=t0[:, hs:hs + hw_], in1=rr, op=ALU.subtract
                    )
                    nc.scalar.activation(
                        emb[:, hs:hs + hw_], ys, AF.Sin, bias=zcol, scale=TWO_PI
                    )
                    nc.sync.dma_start(
                        out=emb[36:39, hs:hs + hw_], in_=ptsh[:, s + hs:s + hs + hw_]
                    )

                # ---- MLP layers ----
                # h tile layout: [128, 4*NT]: A-half0, A-half1, B-half0, B-half1
                h_prev = None
                h3 = None
                for li in range(8):
                    h = hp.tile([128, 4 * NT], F16, tag="h")
                    # chunks: list of (weight tile [128,256], rhs per half_x)
                    if li == 0:
                        chunks = [(w0s, lambda hx: emb[:, bass_ts(hx, NT)])]
                    elif li == 4:
                        chunks = [
                            (w4es, lambda hx: emb[:, bass_ts(hx, NT)]),
                            (w4as, lambda hx, hp3=h3: hp3[:, bass_ts(2 * hx, NT)]),
                            (w4bs, lambda hx, hp3=h3: hp3[:, bass_ts(2 * hx + 1, NT)]),
                        ]
                    else:
                        chunks = [
                            (wks[li][0], lambda hx, hp_=h_prev: hp_[:, bass_ts(2 * hx, NT)]),
                            (wks[li][1], lambda hx, hp_=h_prev: hp_[:, bass_ts(2 * hx + 1, NT)]),
                        ]
                    ps = {(hx, m): pp.tile([128, NT], F32, tag="mm", name="psmm")
                          for hx in range(2) for m in range(2)}
                    last = len(chunks) - 1
                    for hx in range(2):
                        for m in range(2):
                            for ci, (wt, rhs) in enumerate(chunks):
                                nc.tensor.matmul(
                                    ps[(hx, m)], wt[:, bass_ts(m, 128)], rhs(hx),
                                    start=(ci == 0), stop=(ci == last),
                                )
                    # ReLU + bias -> h
                    for half_x in range(2):
                        for m in range(2):
                            dst = h[:, bass_ts(2 * half_x + m, NT)]
                            bias_ap = bms[:, li * 2 + m:li * 2 + m + 1]
                            if DVE_RELU.get((li, m), False):
                                nc.vector.tensor_scalar(
                                    dst, ps[(half_x, m)], bias_ap, 0.0,
                                    op0=ALU.add, op1=ALU.max,
                                )
                            else:
                                nc.scalar.activation(
                                    dst, ps[(half_x, m)], AF.Relu, bias=bias_ap,
                                )
                    if li == 3:
                        h3 = h
                    h_prev = h

                # ---- final SDF layer (M=1): plain PSUM tiles, normal
                # accumulation groups (col-packed tile_position measured
                # 279ns/matmul vs 216ns for this plain form).
                psfa = pf.tile([1, NT], F32, tag="finA", name="psfa")
                psfb = pf.tile([1, NT], F32, tag="finB", name="psfb")
                nc.tensor.matmul(
                    psfa, wsdf_a, h_prev[:, bass_ts(0, NT)], start=True, stop=False
                )
                nc.tensor.matmul(
                    psfb, wsdf_a, h_prev[:, bass_ts(2, NT)], start=True, stop=False
                )
                nc.tensor.matmul(
                    psfa, wsdf_b, h_prev[:, bass_ts(1, NT)], start=False, stop=True
                )
                nc.tensor.matmul(
                    psfb, wsdf_b, h_prev[:, bass_ts(3, NT)], start=False, stop=True
                )
                oa = op_.tile([1, NT], F32, tag="oa")
                nc.scalar.activation(oa, psfa, AF.Identity, bias=bsdfs[0:1, 0:1])
                ob = op_.tile([1, NT], F32, tag="ob")
                nc.scalar.activation(ob, psfb, AF.Identity, bias=bsdfs[0:1, 0:1])
                nc.sync.dma_start(out=out_o[2 * p:2 * p + 1, :], in_=oa)
                nc.sync.dma_start(out=out_o[2 * p + 1:2 * p + 2, :], in_=ob)
    nc.compile()
    return nc


def bass_ts(i, size):
    return slice(i * size, (i + 1) * size)


def _prep_maps(points, ws, bs, wsdf, bsdf):
    pts = np.ascontiguousarray(points, dtype=np.float32).reshape(N, 3)
    freqs = (2.0 ** np.arange(NHARM)).astype(np.float32)
    fcol18 = (np.repeat(freqs[None, :], 3, axis=0).reshape(18, 1) / TWO_PI).astype(
        np.float32
    )

    bmat = np.zeros((128, 16), dtype=np.float32)
    for i in range(8):
        for m in range(2):
            bmat[:, i * 2 + m] = bs[i][m * 128:(m + 1) * 128]

    wpack = np.zeros((128, WCOLS), dtype=np.float16)
    wpack[0:E, OFF_W0:OFF_W0 + 256] = ws[0].astype(np.float16)
    for li in _K_LAYERS:
        wpack[:, OFF_WK[li]:OFF_WK[li] + 256] = ws[li][0:128, :].astype(np.float16)
        wpack[:, OFF_WK[li] + 256:OFF_WK[li] + 512] = ws[li][128:256, :].astype(
            np.float16
        )
    wpack[0:E, OFF_W4E:OFF_W4E + 256] = ws[4][0:E, :].astype(np.float16)
    wpack[:, OFF_W4A:OFF_W4A + 256] = ws[4][E:E + 128, :].astype(np.float16)
    wpack[:, OFF_W4B:OFF_W4B + 256] = ws[4][E + 128:E + 256, :].astype(np.float16)
    wpack[:, OFF_SDF:OFF_SDF + 1] = wsdf[0:128, :].astype(np.float16)
    wpack[:, OFF_SDF + 1:OFF_SDF + 2] = wsdf[128:256, :].astype(np.float16)

    common = {
        "wpack": wpack,
        "wwarm": np.zeros((128, 512), dtype=np.float16),
        "bmat": bmat,
        "bsdf1": np.full((128, 1), float(np.ravel(bsdf)[0]), dtype=np.float32),
    }

    in_maps = []
    for c in range(N_CORES):
        sl = pts[c * NPC:(c + 1) * NPC]  # [NPC, 3]
        ptsT = np.ascontiguousarray(sl.T)  # [3, NPC]
        rep3 = np.repeat(ptsT, NHARM, axis=0)  # [18, NPC]
        t18 = rep3 * fcol18  # x * 2^j / (2pi), exact fp32 scaling
        rep6 = np.zeros((128, NPC), dtype=np.float32)
        rep6[0:18], rep6[18:36] = t18, t18 + np.float32(0.25)
        m = dict(common)
        m["rep6"] = rep6
        m["ptsh"] = ptsT.astype(np.float16)
        in_maps.append(m)
    return in_maps


def kernel(
    points, w0, b0, w1, b1, w2, b2, w3, b3, w4, b4, w5, b5, w6, b6, w7, b7,
    wsdf, bsdf,
):
    ws = [np.asarray(w, dtype=np.float32) for w in (w0, w1, w2, w3, w4, w5, w6, w7)]
    bs = [np.asarray(b, dtype=np.float32) for b in (b0, b1, b2, b3, b4, b5, b6, b7)]
    in_maps = _prep_maps(
        np.asarray(points), ws, bs,
        np.asarray(wsdf, dtype=np.float32), np.asarray(bsdf, dtype=np.float32),
    )

    if "nc" not in _CACHED:
        _CACHED["nc"] = _build()
    nc = _CACHED["nc"]

    res = run_bass_kernel_spmd(nc, in_maps, core_ids=list(range(N_CORES)))
    out = np.concatenate(
        [res.results[c]["out_o"] for c in range(N_CORES)], axis=0
    ).reshape(N, 1).astype(np.float32)
    return out


# revision 10
# speedup vs baseline: 1.0239x; 1.0022x over previous
"""Trainium2 Bass kernel for nn_NeuralSurface (8-layer MLP SDF with harmonic
embedding + skip concat), data-parallel over 8 NeuronCores.

Layout strategy: activations kept transposed in SBUF ([features, points]),
weights stationary fp16, PE matmuls K/M-chunked to 128. Harmonic sin/cos via
ScalarE Sin LUT after DVE range reduction to [-pi, pi] (magic-number
round-to-nearest). ReLU+bias split between ScalarE (activation Relu w/ bias)
and VectorE (tensor_scalar add+max) reading PSUM. n-tiles processed in pairs
so the PE always has independent matmul work while ReLUs complete.

v4: packed weights DMA'd on the idle GpSimd queue; range reduction moved to
the host (rep6 carries frac(x*2^j/2pi) in fp16: 16x less HBM traffic than the
padded-fp32 original, and no DVE rr/ys ops at all); K=39 matmuls for layer 0 /
the layer-4 emb chunk instead of K-padding to 128; emb is two per-half tiles
so the first L0 matmul only waits on half A; ptsh DMAs issued before the sin;
PE warmup matmuls bridge the DVFS p-state ramp.
"""

import numpy as np

import concourse.bacc as bacc
import concourse.mybir as mybir
import concourse.tile as tile
from concourse.bass_utils import run_bass_kernel_spmd

AF = mybir.ActivationFunctionType
ALU = mybir.AluOpType
F32 = mybir.dt.float32
F16 = mybir.dt.float16

N_CORES = 8
N = 262144
NPC = N // N_CORES  # 32768 points per core
NT = 512  # points per n-tile (PSUM bank / fp32 moving-operand limit)
PAIRS = NPC // (2 * NT)  # 32
H = 256
E = 39
NHARM = 6
TWO_PI = float(2.0 * np.pi)
MAGIC = float(1.5 * 2.0**23)  # round-to-nearest via (x + M) - M

# packed weight tensor column offsets ([128, WCOLS] fp16; K on partitions)
OFF_W0 = 0
_K_LAYERS = (1, 2, 3, 5, 6, 7)
OFF_WK = {li: 256 + j * 512 for j, li in enumerate(_K_LAYERS)}  # ka, kb halves
OFF_W4E = 256 + 6 * 512  # 3328
OFF_W4A = OFF_W4E + 256
OFF_W4B = OFF_W4A + 256
OFF_SDF = OFF_W4B + 256  # 2 cols: wsdf K-halves a, b
WCOLS = OFF_SDF + 2  # 4098

N_WARMUP = 6  # PE p-state ramp matmuls before real work

# ReLU engine split: half 0 -> ACT, half 1 -> DVE (even split; each PSUM
# pair drains through two engines in parallel).
DVE_RELU = {(li, 1): True for li in range(8)}

_CACHED = {}


def _build():
    nc = bacc.Bacc("TRN2")

    rep6 = nc.dram_tensor("rep6", [36, NPC], F16, kind="ExternalInput").ap()
    ptsh = nc.dram_tensor("ptsh", [3, NPC], F16, kind="ExternalInput").ap()
    wpack = nc.dram_tensor("wpack", [128, WCOLS], F16, kind="ExternalInput").ap()
    wwarm = nc.dram_tensor("wwarm", [128, 512], F16, kind="ExternalInput").ap()
    bmat = nc.dram_tensor("bmat", [128, 16], F32, kind="ExternalInput").ap()
    bsdf1 = nc.dram_tensor("bsdf1", [128, 1], F32, kind="ExternalInput").ap()
    # 2-D output (1-D ExternalOutput tensors fail NEFF load under bass2jax)
    out_o = nc.dram_tensor("out_o", [NPC // NT, NT], F32, kind="ExternalOutput").ap()

    with tile.TileContext(nc) as tc:
        with (
            tc.tile_pool(name="wp", bufs=1) as wp,
            tc.tile_pool(name="ep", bufs=4) as ep,
            tc.tile_pool(name="hp", bufs=4) as hp,
            tc.tile_pool(name="op", bufs=4) as op_,
            tc.tile_pool(name="pp", bufs=6, space="PSUM") as pp,
            tc.tile_pool(name="pf", bufs=1, space="PSUM") as pf,
        ):
            # ---- one-time weight / const loads ----
            # warmup weights on the scalar queue, packed weights in 3 chunks
            # on the gpsimd queue: both idle at boot, so the sync queue's
            # first issue is pair-0's rep6 (the embedding critical path).
            wws = wp.tile([128, 512], F16, name="wws")
            nc.scalar.dma_start(out=wws, in_=wwarm)
            wps = wp.tile([128, WCOLS], F16, name="wps")
            nc.gpsimd.dma_start(out=wps[:, 0:512], in_=wpack[:, 0:512])
            nc.gpsimd.dma_start(out=wps[:, 512:2304], in_=wpack[:, 512:2304])
            nc.gpsimd.dma_start(out=wps[:, 2304:WCOLS], in_=wpack[:, 2304:WCOLS])
            bms = wp.tile_from(bmat, name="bms")  # [128, 16]
            bsdfs = wp.tile_from(bsdf1, name="bsdfs")  # [1, 1]
            zcol = wp.tile([128, 1], F32, name="zcol")
            nc.vector.memset(zcol, 0.0)

            w0s = wps[0:E, OFF_W0:OFF_W0 + 256]  # [39, 256]
            wks = {
                li: (
                    wps[:, OFF_WK[li]:OFF_WK[li] + 256],
                    wps[:, OFF_WK[li] + 256:OFF_WK[li] + 512],
                )
                for li in _K_LAYERS
            }
            w4es = wps[0:E, OFF_W4E:OFF_W4E + 256]  # [39, 256]
            w4as = wps[:, OFF_W4A:OFF_W4A + 256]
            w4bs = wps[:, OFF_W4B:OFF_W4B + 256]
            wsdf_a = wps[:, OFF_SDF:OFF_SDF + 1]  # [128, 1]
            wsdf_b = wps[:, OFF_SDF + 1:OFF_SDF + 2]

            # ---- PE p-state warmup: dummy matmuls, no consumers ----
            # shares the "finA" PSUM bank (pair-0's SDF writes it much later)
            pwt = pf.tile([128, NT], F32, tag="finA", name="pwt")
            for _ in range(N_WARMUP):
                nc.tensor.matmul(pwt, wws[:, 0:128], wws, start=True, stop=True)

            for p in range(PAIRS):
                s = p * 2 * NT  # start point index of the pair (A at s, B at s+NT)
                W = 2 * NT  # pair-wide free size

                # ---- embedding ----
                # rep6 rows carry y = frac(x*2^j/(2pi) + phase) in [-.5, .5]
                # (host-side fp32 range reduction; 18 sin rows + 18
                # cos-as-phase-shift rows), fp16. emb is built as two [39, NT]
                # half tiles so each L0 matmul waits only on its own half;
                # ptsh (rows 36:39) lands before the sin (disjoint rows).
                t0 = ep.tile([36, W], F16, tag="t0")
                nc.sync.dma_start(out=t0, in_=rep6[:, s:s + W])
                emb_a = ep.tile([E, NT], F16, tag="embA", name="emb_a")
                emb_b = ep.tile([E, NT], F16, tag="embB", name="emb_b")
                embh = (emb_a, emb_b)
                for hx in range(2):
                    nc.sync.dma_start(
                        out=embh[hx][36:39, :],
                        in_=ptsh[:, s + hx * NT:s + (hx + 1) * NT],
                    )
                for hx in range(2):
                    hs = hx * NT
                    nc.scalar.activation(
                        embh[hx][0:36, :], t0[:, hs:hs + NT], AF.Sin,
                        bias=zcol[0:36, :], scale=TWO_PI,
                    )

                # ---- MLP layers ----
                # h tile layout: [128, 4*NT]: A-half0, A-half1, B-half0, B-half1
                h_prev = None
                h3 = None
                for li in range(8):
                    h = hp.tile([128, 4 * NT], F16, tag="h")
                    # chunks: list of (weight AP, rhs per half_x)
                    if li == 0:
                        chunks = [(w0s, lambda hx: embh[hx])]
                    elif li == 4:
                        chunks = [
                            (w4es, lambda hx: embh[hx]),
                            (w4as, lambda hx, hp3=h3: hp3[:, bass_ts(2 * hx, NT)]),
                            (w4bs, lambda hx, hp3=h3: hp3[:, bass_ts(2 * hx + 1, NT)]),
                        ]
                    else:
                        chunks = [
                            (wks[li][0], lambda hx, hp_=h_prev: hp_[:, bass_ts(2 * hx, NT)]),
                            (wks[li][1], lambda hx, hp_=h_prev: hp_[:, bass_ts(2 * hx + 1, NT)]),
                        ]
                    ps = {(hx, m): pp.tile([128, NT], F32, tag="mm", name="psmm")
                          for hx in range(2) for m in range(2)}
                    last = len(chunks) - 1
                    for hx in range(2):
                        for m in range(2):
                            for ci, (wt, rhs) in enumerate(chunks):
                                nc.tensor.matmul(
                                    ps[(hx, m)], wt[:, bass_ts(m, 128)], rhs(hx),
                                    start=(ci == 0), stop=(ci == last),
                                )
                    # ReLU + bias -> h
                    for half_x in range(2):
                        for m in range(2):
                            dst = h[:, bass_ts(2 * half_x + m, NT)]
                            bias_ap = bms[:, li * 2 + m:li * 2 + m + 1]
                            if DVE_RELU.get((li, m), False):
                                nc.vector.tensor_scalar(
                                    dst, ps[(half_x, m)], bias_ap, 0.0,
                                    op0=ALU.add, op1=ALU.max,
                                )
                            else:
                                nc.scalar.activation(
                                    dst, ps[(half_x, m)], AF.Relu, bias=bias_ap,
                                )
                    if li == 3:
                        h3 = h
                    h_prev = h

                # ---- final SDF layer (M=1), plain PSUM tiles ----
                psfa = pf.tile([1, NT], F32, tag="finA", name="psfa")
                psfb = pf.tile([1, NT], F32, tag="finB", name="psfb")
                nc.tensor.matmul(
                    psfa, wsdf_a, h_prev[:, bass_ts(0, NT)], start=True, stop=False
                )
                nc.tensor.matmul(
                    psfb, wsdf_a, h_prev[:, bass_ts(2, NT)], start=True, stop=False
                )
                nc.tensor.matmul(
                    psfa, wsdf_b, h_prev[:, bass_ts(1, NT)], start=False, stop=True
                )
                nc.tensor.matmul(
                    psfb, wsdf_b, h_prev[:, bass_ts(3, NT)], start=False, stop=True
                )
                oa = op_.tile([1, NT], F32, tag="oa")
                nc.scalar.activation(oa, psfa, AF.Identity, bias=bsdfs[0:1, 0:1])
                ob = op_.tile([1, NT], F32, tag="ob")
                nc.scalar.activation(ob, psfb, AF.Identity, bias=bsdfs[0:1, 0:1])
                nc.sync.dma_start(out=out_o[2 * p:2 * p + 1, :], in_=oa)
                nc.sync.dma_start(out=out_o[2 * p + 1:2 * p + 2, :], in_=ob)
    nc.compile()
    return nc


def bass_ts(i, size):
    return slice(i * size, (i + 1) * size)


def _prep_maps(points, ws, bs, wsdf, bsdf):
    pts = np.ascontiguousarray(points, dtype=np.float32).reshape(N, 3)
    freqs = (2.0 ** np.arange(NHARM)).astype(np.float32)
    fcol18 = (np.repeat(freqs[None, :], 3, axis=0).reshape(18, 1) / TWO_PI).astype(
        np.float32
    )

    bmat = np.zeros((128, 16), dtype=np.float32)
    for i in range(8):
        for m in range(2):
            bmat[:, i * 2 + m] = bs[i][m * 128:(m + 1) * 128]

    wpack = np.zeros((128, WCOLS), dtype=np.float16)
    wpack[0:E, OFF_W0:OFF_W0 + 256] = ws[0].astype(np.float16)
    for li in _K_LAYERS:
        wpack[:, OFF_WK[li]:OFF_WK[li] + 256] = ws[li][0:128, :].astype(np.float16)
        wpack[:, OFF_WK[li] + 256:OFF_WK[li] + 512] = ws[li][128:256, :].astype(
            np.float16
        )
    wpack[0:E, OFF_W4E:OFF_W4E + 256] = ws[4][0:E, :].astype(np.float16)
    wpack[:, OFF_W4A:OFF_W4A + 256] = ws[4][E:E + 128, :].astype(np.float16)
    wpack[:, OFF_W4B:OFF_W4B + 256] = ws[4][E + 128:E + 256, :].astype(np.float16)
    wpack[:, OFF_SDF:OFF_SDF + 1] = wsdf[0:128, :].astype(np.float16)
    wpack[:, OFF_SDF + 1:OFF_SDF + 2] = wsdf[128:256, :].astype(np.float16)

    common = {
        "wpack": wpack,
        "wwarm": np.zeros((128, 512), dtype=np.float16),
        "bmat": bmat,
        "bsdf1": np.full((128, 1), float(np.ravel(bsdf)[0]), dtype=np.float32),
    }

    in_maps = []
    for c in range(N_CORES):
        sl = pts[c * NPC:(c + 1) * NPC]  # [NPC, 3]
        ptsT = np.ascontiguousarray(sl.T)  # [3, NPC]
        rep3 = np.repeat(ptsT, NHARM, axis=0)  # [18, NPC]
        t18 = rep3 * fcol18  # x * 2^j / (2pi), exact fp32 scaling
        t36 = np.empty((36, NPC), dtype=np.float32)
        t36[0:18], t36[18:36] = t18, t18 + np.float32(0.25)
        # host-side range reduction to [-0.5, 0.5] turns (same fp32 math the
        # kernel's DVE magic-round did); Sin LUT sees scale*y in [-pi, pi]
        rep6 = (t36 - np.round(t36)).astype(np.float16)
        m = dict(common)
        m["rep6"] = rep6
        m["ptsh"] = ptsT.astype(np.float16)
        in_maps.append(m)
    return in_maps


def kernel(
    points, w0, b0, w1, b1, w2, b2, w3, b3, w4, b4, w5, b5, w6, b6, w7, b7,
    wsdf, bsdf,
):
    ws = [np.asarray(w, dtype=np.float32) for w in (w0, w1, w2, w3, w4, w5, w6, w7)]
    bs = [np.asarray(b, dtype=np.float32) for b in (b0, b1, b2, b3, b4, b5, b6, b7)]
    in_maps = _prep_maps(
        np.asarray(points), ws, bs,
        np.asarray(wsdf, dtype=np.float32), np.asarray(bsdf, dtype=np.float32),
    )

    if "nc" not in _CACHED:
        _CACHED["nc"] = _build()
    nc = _CACHED["nc"]

    res = run_bass_kernel_spmd(nc, in_maps, core_ids=list(range(N_CORES)))
    out = np.concatenate(
        [res.results[c]["out_o"] for c in range(N_CORES)], axis=0
    ).reshape(N, 1).astype(np.float32)
    return out


# revision 12
# speedup vs baseline: 1.0849x; 1.0596x over previous
"""Trainium2 Bass kernel for nn_NeuralSurface (8-layer MLP SDF with harmonic
embedding + skip concat), data-parallel over 8 NeuronCores.

Layout strategy: activations kept transposed in SBUF ([features, points]),
weights stationary fp16, PE matmuls K/M-chunked to 128. Harmonic sin/cos via
ScalarE Sin LUT (range reduction done host-side). ReLU+bias split between
ScalarE (activation Relu w/ bias) and VectorE (tensor_scalar add+max) reading
PSUM. n-tiles processed in pairs so the PE always has independent matmul work
while ReLUs complete.

v5: rep6 carries frac(x*2^j/2pi) in fp16 (host range reduction; 16x less HBM
traffic than padded-fp32, no DVE rr/ys ops); emb K-padded to 128 via GpSimd
memsets (K<128 matmuls measured +80ns each); packed weights DMA'd in 3 chunks
on the scalar queue; a dummy Sin preloads the trig LUT set before the real
chain needs it; each pair's SDF matmuls are emitted after the NEXT pair's L0
block to deepen the two thinnest relu->matmul shadows; SDF bias/copy runs on
the slack-rich VectorE instead of ScalarE.
"""

import numpy as np

import concourse.bacc as bacc
import concourse.mybir as mybir
import concourse.tile as tile
from concourse.bass_utils import run_bass_kernel_spmd

AF = mybir.ActivationFunctionType
ALU = mybir.AluOpType
F32 = mybir.dt.float32
F16 = mybir.dt.float16

N_CORES = 8
N = 262144
NPC = N // N_CORES  # 32768 points per core
NT = 512  # points per n-tile (PSUM bank / fp32 moving-operand limit)
PAIRS = NPC // (2 * NT)  # 32
H = 256
E = 39
NHARM = 6
TWO_PI = float(2.0 * np.pi)

# packed weight tensor column offsets ([128, WCOLS] fp16; K on partitions)
OFF_W0 = 0
_K_LAYERS = (1, 2, 3, 5, 6, 7)
OFF_WK = {li: 256 + j * 512 for j, li in enumerate(_K_LAYERS)}  # ka, kb halves
OFF_W4E = 256 + 6 * 512  # 3328
OFF_W4A = OFF_W4E + 256
OFF_W4B = OFF_W4A + 256
OFF_SDF = OFF_W4B + 256  # 2 cols: wsdf K-halves a, b
WCOLS = OFF_SDF + 2  # 4098

# ReLU engine split: half 0 -> ACT, half 1 -> DVE (even split; each PSUM
# pair drains through two engines in parallel).
DVE_RELU = {(li, 1): True for li in range(8)}

_CACHED = {}


def _build():
    nc = bacc.Bacc("TRN2")

    rep6 = nc.dram_tensor("rep6", [36, NPC], F16, kind="ExternalInput").ap()
    ptsh = nc.dram_tensor("ptsh", [3, NPC], F16, kind="ExternalInput").ap()
    wpack = nc.dram_tensor("wpack", [128, WCOLS], F16, kind="ExternalInput").ap()
    bmat = nc.dram_tensor("bmat", [128, 16], F32, kind="ExternalInput").ap()
    bsdf1 = nc.dram_tensor("bsdf1", [128, 1], F32, kind="ExternalInput").ap()
    # 2-D output (1-D ExternalOutput tensors fail NEFF load under bass2jax)
    out_o = nc.dram_tensor("out_o", [NPC // NT, NT], F32, kind="ExternalOutput").ap()

    with tile.TileContext(nc) as tc:
        with (
            tc.tile_pool(name="wp", bufs=1) as wp,
            tc.tile_pool(name="ep", bufs=4) as ep,
            tc.tile_pool(name="hp", bufs=4) as hp,
            tc.tile_pool(name="op", bufs=4) as op_,
            tc.tile_pool(name="pp", bufs=6, space="PSUM") as pp,
            tc.tile_pool(name="pf", bufs=1, space="PSUM") as pf,
        ):
            # ---- one-time weight / const loads ----
            # packed weights in 3 chunks on the scalar queue (idle at boot;
            # issues run during the ACT table load), so the sync queue's
            # first issue is pair-0's rep6 (the embedding critical path) and
            # the gpsimd queue only carries the per-pair emb pad memsets.
            zcol = wp.tile([128, 1], F32, name="zcol")
            nc.vector.memset(zcol, 0.0)
            # dummy Sin: forces the trig_and_small LUT set (which also holds
            # Relu/Identity) to load once at boot instead of right before
            # pair-0's sin.
            sindum = wp.tile([36, 1], F16, name="sindum")
            nc.scalar.activation(sindum, zcol[0:36, :], AF.Sin, bias=zcol[0:36, :])
            wps = wp.tile([128, WCOLS], F16, name="wps")
            nc.scalar.dma_start(out=wps[:, 0:512], in_=wpack[:, 0:512])
            nc.scalar.dma_start(out=wps[:, 512:2304], in_=wpack[:, 512:2304])
            nc.scalar.dma_start(out=wps[:, 2304:WCOLS], in_=wpack[:, 2304:WCOLS])
            bms = wp.tile_from(bmat, name="bms")  # [128, 16]
            bsdfs = wp.tile_from(bsdf1, name="bsdfs")  # [1, 1]

            w0s = wps[:, OFF_W0:OFF_W0 + 256]  # [128(39), 256]
            wks = {
                li: (
                    wps[:, OFF_WK[li]:OFF_WK[li] + 256],
                    wps[:, OFF_WK[li] + 256:OFF_WK[li] + 512],
                )
                for li in _K_LAYERS
            }
            w4es = wps[:, OFF_W4E:OFF_W4E + 256]
            w4as = wps[:, OFF_W4A:OFF_W4A + 256]
            w4bs = wps[:, OFF_W4B:OFF_W4B + 256]
            wsdf_a = wps[:, OFF_SDF:OFF_SDF + 1]  # [128, 1]
            wsdf_b = wps[:, OFF_SDF + 1:OFF_SDF + 2]

            # previous pair's state for the deferred SDF emission
            h7_prev = None
            psf_prev = None

            def emit_sdf(h7):
                psfa = pf.tile([1, NT], F32, tag="finA", name="psfa")
                psfb = pf.tile([1, NT], F32, tag="finB", name="psfb")
                nc.tensor.matmul(
                    psfa, wsdf_a, h7[:, bass_ts(0, NT)], start=True, stop=False
                )
                nc.tensor.matmul(
                    psfb, wsdf_a, h7[:, bass_ts(2, NT)], start=True, stop=False
                )
                nc.tensor.matmul(
                    psfa, wsdf_b, h7[:, bass_ts(1, NT)], start=False, stop=True
                )
                nc.tensor.matmul(
                    psfb, wsdf_b, h7[:, bass_ts(3, NT)], start=False, stop=True
                )
                return psfa, psfb

            def emit_sdf_out(pq, psfa, psfb):
                # bias-add + PSUM->SBUF on VectorE (ScalarE is the busier
                # engine: sins + half the relus)
                oa = op_.tile([1, NT], F32, tag="oa")
                nc.vector.tensor_scalar(
                    oa, psfa, bsdfs[0:1, 0:1], None, op0=ALU.add
                )
                ob = op_.tile([1, NT], F32, tag="ob")
                nc.vector.tensor_scalar(
                    ob, psfb, bsdfs[0:1, 0:1], None, op0=ALU.add
                )
                nc.sync.dma_start(out=out_o[2 * pq:2 * pq + 1, :], in_=oa)
                nc.sync.dma_start(out=out_o[2 * pq + 1:2 * pq + 2, :], in_=ob)

            for p in range(PAIRS):
                s = p * 2 * NT  # start point index of the pair (A at s, B at s+NT)
                W = 2 * NT  # pair-wide free size

                # ---- embedding ----
                # rep6 rows carry y = frac(x*2^j/(2pi) + phase) in [-.5, .5]
                # (host-side fp32 range reduction; 18 sin rows + 18
                # cos-as-phase-shift rows), fp16. emb is two [128, NT] half
                # tiles (rows 39:128 zeroed on GpSimd so L0 runs full-K
                # matmuls; K<128 measured +80ns per matmul) so each L0
                # matmul waits only on its own half; ptsh (rows 36:39) lands
                # independent of the sin (disjoint rows).
                t0 = ep.tile([36, W], F16, tag="t0")
                nc.sync.dma_start(out=t0, in_=rep6[:, s:s + W])
                emb_a = ep.tile([128, NT], F16, tag="embA", name="emb_a")
                emb_b = ep.tile([128, NT], F16, tag="embB", name="emb_b")
                embh = (emb_a, emb_b)
                for hx in range(2):
                    # full-tile memset (GPSIMD partition access must start at
                    # 0); sin/ptsh overwrite rows 0:39 afterwards
                    nc.gpsimd.memset(embh[hx], 0.0)
                    nc.sync.dma_start(
                        out=embh[hx][36:39, :],
                        in_=ptsh[:, s + hx * NT:s + (hx + 1) * NT],
                    )
                    nc.scalar.activation(
                        embh[hx][0:36, :], t0[:, hx * NT:(hx + 1) * NT], AF.Sin,
                        bias=zcol[0:36, :], scale=TWO_PI,
                    )

                # ---- MLP layers ----
                # h tile layout: [128, 4*NT]: A-half0, A-half1, B-half0, B-half1
                h3 = None
                h_prev = None
                for li in range(8):
                    h = hp.tile([128, 4 * NT], F16, tag="h")
                    # chunks: list of (weight AP, rhs per half_x)
                    if li == 0:
                        chunks = [(w0s, lambda hx: embh[hx])]
                    elif li == 4:
                        chunks = [
                            (w4es, lambda hx: embh[hx]),
                            (w4as, lambda hx, hp3=h3: hp3[:, bass_ts(2 * hx, NT)]),
                            (w4bs, lambda hx, hp3=h3: hp3[:, bass_ts(2 * hx + 1, NT)]),
                        ]
                    else:
                        chunks = [
                            (wks[li][0], lambda hx, hp_=h_prev: hp_[:, bass_ts(2 * hx, NT)]),
                            (wks[li][1], lambda hx, hp_=h_prev: hp_[:, bass_ts(2 * hx + 1, NT)]),
                        ]
                    ps = {(hx, m): pp.tile([128, NT], F32, tag="mm", name="psmm")
                          for hx in range(2) for m in range(2)}
                    last = len(chunks) - 1
                    for hx in range(2):
                        for m in range(2):
                            for ci, (wt, rhs) in enumerate(chunks):
                                nc.tensor.matmul(
                                    ps[(hx, m)], wt[:, bass_ts(m, 128)], rhs(hx),
                                    start=(ci == 0), stop=(ci == last),
                                )
                    # ReLU + bias -> h
                    for half_x in range(2):
                        for m in range(2):
                            dst = h[:, bass_ts(2 * half_x + m, NT)]
                            bias_ap = bms[:, li * 2 + m:li * 2 + m + 1]
                            if DVE_RELU.get((li, m), False):
                                nc.vector.tensor_scalar(
                                    dst, ps[(half_x, m)], bias_ap, 0.0,
                                    op0=ALU.add, op1=ALU.max,
                                )
                            else:
                                nc.scalar.activation(
                                    dst, ps[(half_x, m)], AF.Relu, bias=bias_ap,
                                )
                    if li == 0 and h7_prev is not None:
                        # previous pair's SDF matmuls slot in here: they are
                        # ready to run (h7 relus done) and deepen both the
                        # L0->L1 and L7->SDF relu shadows by 4 matmuls.
                        psf_prev = emit_sdf(h7_prev)
                        h7_prev = None
                    if li == 3:
                        h3 = h
                        if psf_prev is not None:
                            emit_sdf_out(p - 1, *psf_prev)
                            psf_prev = None
                    h_prev = h

                h7_prev = h_prev

            psfa, psfb = emit_sdf(h7_prev)
            emit_sdf_out(PAIRS - 1, psfa, psfb)
    nc.compile()
    return nc


def bass_ts(i, size):
    return slice(i * size, (i + 1) * size)


def _prep_maps(points, ws, bs, wsdf, bsdf):
    pts = np.ascontiguousarray(points, dtype=np.float32).reshape(N, 3)
    freqs = (2.0 ** np.arange(NHARM)).astype(np.float32)
    fcol18 = (np.repeat(freqs[None, :], 3, axis=0).reshape(18, 1) / TWO_PI).astype(
        np.float32
    )

    bmat = np.zeros((128, 16), dtype=np.float32)
    for i in range(8):
        for m in range(2):
            bmat[:, i * 2 + m] = bs[i][m * 128:(m + 1) * 128]

    wpack = np.zeros((128, WCOLS), dtype=np.float16)
    wpack[0:E, OFF_W0:OFF_W0 + 256] = ws[0].astype(np.float16)
    for li in _K_LAYERS:
        wpack[:, OFF_WK[li]:OFF_WK[li] + 256] = ws[li][0:128, :].astype(np.float16)
        wpack[:, OFF_WK[li] + 256:OFF_WK[li] + 512] = ws[li][128:256, :].astype(
            np.float16
        )
    wpack[0:E, OFF_W4E:OFF_W4E + 256] = ws[4][0:E, :].astype(np.float16)
    wpack[:, OFF_W4A:OFF_W4A + 256] = ws[4][E:E + 128, :].astype(np.float16)
    wpack[:, OFF_W4B:OFF_W4B + 256] = ws[4][E + 128:E + 256, :].astype(np.float16)
    wpack[:, OFF_SDF:OFF_SDF + 1] = wsdf[0:128, :].astype(np.float16)
    wpack[:, OFF_SDF + 1:OFF_SDF + 2] = wsdf[128:256, :].astype(np.float16)

    common = {
        "wpack": wpack,
        "bmat": bmat,
        "bsdf1": np.full((128, 1), float(np.ravel(bsdf)[0]), dtype=np.float32),
    }

    in_maps = []
    for c in range(N_CORES):
        sl = pts[c * NPC:(c + 1) * NPC]  # [NPC, 3]
        ptsT = np.ascontiguousarray(sl.T)  # [3, NPC]
        rep3 = np.repeat(ptsT, NHARM, axis=0)  # [18, NPC]
        t18 = rep3 * fcol18  # x * 2^j / (2pi), exact fp32 scaling
        t36 = np.empty((36, NPC), dtype=np.float32)
        t36[0:18], t36[18:36] = t18, t18 + np.float32(0.25)
        # host-side range reduction to [-0.5, 0.5] turns (same fp32 math the
        # kernel's DVE magic-round did); Sin LUT sees scale*y in [-pi, pi]
        rep6 = (t36 - np.round(t36)).astype(np.float16)
        m = dict(common)
        m["rep6"] = rep6
        m["ptsh"] = ptsT.astype(np.float16)
        in_maps.append(m)
    return in_maps


def kernel(
    points, w0, b0, w1, b1, w2, b2, w3, b3, w4, b4, w5, b5, w6, b6, w7, b7,
    wsdf, bsdf,
):
    ws = [np.asarray(w, dtype=np.float32) for w in (w0, w1, w2, w3, w4, w5, w6, w7)]
    bs = [np.asarray(b, dtype=np.float32) for b in (b0, b1, b2, b3, b4, b5, b6, b7)]
    in_maps = _prep_maps(
        np.asarray(points), ws, bs,
        np.asarray(wsdf, dtype=np.float32), np.asarray(bsdf, dtype=np.float32),
    )

    if "nc" not in _CACHED:
        _CACHED["nc"] = _build()
    nc = _CACHED["nc"]

    res = run_bass_kernel_spmd(nc, in_maps, core_ids=list(range(N_CORES)))
    out = np.concatenate(
        [res.results[c]["out_o"] for c in range(N_CORES)], axis=0
    ).reshape(N, 1).astype(np.float32)
    return out
